# revision 56
# baseline (speedup 1.0000x reference)
"""Trainium2 Bass kernel: multi-head attention (B=2, T=2048, E=1024, H=8, D=512),
bias-free QKV/O projections + RoPE + causal softmax.

Sharding: head-parallel across 8 NeuronCores. Core h computes head h fully;
host sums the 8 partial o_proj outputs (the all-reduce after o_proj).

v2 layout (vs 452us baseline):
  - x / Wq / Wk / Wv / qT / kT / Wo / at_sb in bf16 (same 1 cycle/row on PE,
    half DMA + SBUF); v / probs / rowsum stay f32r for accuracy.
  - attention at 256-wide q tiles (2m+2 causal k-chunks of 128) instead of
    512-wide (4n+4): less masked-diagonal waste on the PE.
  - rowsum via DVE accumulation of exp tiles (S += ex) + ONE 256-row
    ones-matmul per q tile instead of a 512-row matmul per chunk.
  - PE never idles: batch-1 x / cos / sin prefetched during batch-0
    attention (the HAM duty-cycle drops 8/8 -> 4/8 on any PE idle gap and
    costs ~14us to recover); warmup matmuls bridge the startup DMA.
"""
from contextlib import ExitStack

import numpy as np

B, T, E, H, D = 2, 2048, 1024, 8, 512
NTOK = B * T
SCALE = float(1.0 / np.sqrt(D))
NEG = -1.0e30
ROPE_BASE = 10000.0
QT = 256          # attention q-tile width
NQT = T // QT     # 8 q tiles per batch
WARM = 12         # warmup matmuls (512 rows each) bridging startup DMA

PROFILE = False          # set True (e.g. from test.py) to trace core 0
LAST_RESULTS = None      # BassKernelResults of the last run when PROFILE

_CACHE = {}


def _build():
    import concourse.tile as tile
    from concourse import bacc, mybir

    f32 = mybir.dt.float32
    f32r = mybir.dt.float32r
    bf16 = mybir.dt.bfloat16
    AF = mybir.ActivationFunctionType

    nc = bacc.Bacc("TRN2", target_bir_lowering=False, debug=False,
                   enable_asserts=False, num_devices=8)
    xT_d = nc.dram_tensor("xT", [E, NTOK], bf16, kind="ExternalInput").ap()
    wqT_d = nc.dram_tensor("wqT", [E, D], bf16, kind="ExternalInput").ap()
    wkT_d = nc.dram_tensor("wkT", [E, D], bf16, kind="ExternalInput").ap()
    wvT_d = nc.dram_tensor("wvT", [E, D], bf16, kind="ExternalInput").ap()
    woT_d = nc.dram_tensor("woT", [D, E], bf16, kind="ExternalInput").ap()
    cos_d = nc.dram_tensor("cosdt", [D // 2, T], f32, kind="ExternalInput").ap()
    sin_d = nc.dram_tensor("sindt", [D // 2, T], f32, kind="ExternalInput").ap()
    msk_d = nc.dram_tensor("masks", [128, 384], f32, kind="ExternalInput").ap()
    out_d = nc.dram_tensor("out", [NTOK, E], f32, kind="ExternalOutput").ap()

    xT_r = xT_d.rearrange("(eo p) t -> p eo t", p=128)     # [128, 8, 4096]
    cos_r = cos_d.rearrange("(fo p) t -> p fo t", p=128)   # [128, 2, 2048]
    sin_r = sin_d.rearrange("(fo p) t -> p fo t", p=128)
    wq_r = wqT_d.rearrange("(eo p) d -> p eo d", p=128)
    wk_r = wkT_d.rearrange("(eo p) d -> p eo d", p=128)
    wv_r = wvT_d.rearrange("(eo p) d -> p eo d", p=128)
    wo_r = woT_d.rearrange("(do p) e -> p do e", p=128)

    with tile.TileContext(nc) as tc, ExitStack() as top:
        wp = top.enter_context(tc.tile_pool(name="wp", bufs=1))
        wq_t = wp.tile([128, 8, D], bf16, tag="wq", name="wq")
        wk_t = wp.tile([128, 8, D], bf16, tag="wk", name="wk")
        wv_t = wp.tile([128, 8, D], bf16, tag="wv", name="wv")
        wv = [wv_t[:, e] for e in range(8)]
        wo_t = wp.tile([128, 4, E], bf16, tag="wo", name="wo")
        wo = [wo_t[:, d] for d in range(4)]
        mks = wp.tile([128, 384], f32, tag="mks", name="mks")
        mk256 = mks[:, 0:QT]     # diag chunk 2m mask over a full 256-q tile
        mk128 = mks[:, QT:384]   # diag chunk 2m+1 mask over its live 128-q half
        # bf16: the rowsum-transpose matmuls have a 1-wide moving dim (fp32r
        # forbids that) and bf16 avoids a PE mode switch mid-stream.
        onescol = wp.tile([128, 1], bf16, tag="onescol", name="onescol")

        # x tiles: one rolling pool across both batches so batch-1 tiles can
        # be prefetched (DMA'd) while batch-0 attention runs.
        xp = top.enter_context(tc.tile_pool(name="xp", bufs=4))
        csp = top.enter_context(tc.tile_pool(name="csp", bufs=2))

        xts = {}   # (b, tt) -> tile
        css = {}   # (b, tt) -> (cs, sn)

        def issue_x_dma(b, tt):
            t = xp.tile([128, 8, 512], bf16, tag="xt", name="xt")
            g0 = b * T + tt * 512
            nc.sync.dma_start(t[:], xT_r[:, :, g0:g0 + 512])
            xts[(b, tt)] = t

        def issue_cs_dma(b, tt):
            s0 = tt * 512
            cs = csp.tile([128, 2, 512], f32, tag="cs", name="cs")
            sn = csp.tile([128, 2, 512], f32, tag="sn", name="sn")
            nc.sync.dma_start(cs[:], cos_r[:, :, s0:s0 + 512])
            nc.sync.dma_start(sn[:], sin_r[:, :, s0:s0 + 512])
            css[(b, tt)] = (cs, sn)

        for b in range(B):
            tok0 = b * T
            with ExitStack() as bctx:
                qkv = bctx.enter_context(tc.tile_pool(name="qkv", bufs=1))
                qT_t = [qkv.tile([128, T], bf16, tag=f"qT{d}", name=f"qT{d}") for d in range(4)]
                kT_t = [qkv.tile([128, T], bf16, tag=f"kT{d}", name=f"kT{d}") for d in range(4)]
                vv_chunks = [qkv.tile([128, D], bf16, tag=f"v{t}", name=f"v{t}")
                             for t in range(16)]

                # ----- projection phase: qT/kT (RoPE'd) and v -----
                with ExitStack() as pctx:
                    tp = pctx.enter_context(tc.tile_pool(name="tp", bufs=4))
                    pp = pctx.enter_context(
                        tc.tile_pool(name="pp", bufs=6, space="PSUM"))
                    ppv = pctx.enter_context(
                        tc.tile_pool(name="ppv", bufs=2, space="PSUM"))

                    if b == 0:
                        # warmup: PE-busy filler while startup DMA streams in;
                        # lifts the HAM clock gate to 8/8 and produces the
                        # `ones` tile (WARM accumulated ones.T@ones passes).
                        warmp = pctx.enter_context(
                            tc.tile_pool(name="warmp", bufs=1))
                        onef = warmp.tile([128, 128], f32, tag="onef", name="onef")
                        nc.vector.memset(onef[:], 1.0)
                        ones0 = warmp.tile([128, 128], f32r, tag="ones0", name="ones0")
                        nc.vector.tensor_copy(ones0[:], onef[:])
                        nc.vector.memset(onescol[:], 1.0)
                        wsf = warmp.tile([128, 512], f32, tag="wsf", name="wsf")
                        nc.vector.memset(wsf[:], 1.0)
                        wsrc = warmp.tile([128, 512], f32r, tag="wsrc", name="wsrc")
                        nc.vector.tensor_copy(wsrc[:], wsf[:])
                        warm_ps = pp.tile([128, 512], f32, tag="pp", name="pp")
                        for w in range(WARM):
                            nc.tensor.matmul(warm_ps[:], ones0[:], wsrc[:],
                                             start=(w == 0), stop=(w == WARM - 1))
                        # touch Exp so its ACT table set loads during the
                        # DMA-bound startup instead of at the first score tile
                        expre = warmp.tile([128, 1], f32, tag="expre", name="expre")
                        nc.scalar.activation(expre[:], warm_ps[:, :1], AF.Exp,
                                             scale=0.001)
                        nc.vector.tensor_copy(expre[:], expre[:])

                    for tt in range(4):
                        s0 = tt * 512
                        if b == 0:
                            if tt == 0:
                                # need-ordered startup loads, halved so the
                                # first v matmuls (xt e0-3 + wv e0-3) start
                                # as early as possible.
                                t = xp.tile([128, 8, 512], bf16, tag="xt", name="xt")
                                nc.sync.dma_start(t[:, 0:4], xT_r[:, 0:4, 0:512])
                                nc.sync.dma_start(wv_t[:, 0:4], wv_r[:, 0:4])
                                nc.sync.dma_start(wq_t[:, 0:4], wq_r[:, 0:4])
                                nc.sync.dma_start(t[:, 4:8], xT_r[:, 4:8, 0:512])
                                nc.sync.dma_start(wv_t[:, 4:8], wv_r[:, 4:8])
                                nc.sync.dma_start(wq_t[:, 4:8], wq_r[:, 4:8])
                                xts[(0, 0)] = t
                                issue_cs_dma(0, 0)
                                nc.sync.dma_start(mks[:], msk_d)
                                nc.sync.dma_start(wk_t[:], wk_r)
                            else:
                                issue_x_dma(0, tt)
                                issue_cs_dma(0, tt)
                                if tt == 1:
                                    nc.sync.dma_start(wo_t[:], wo_r)
                        else:
                            # batch 1: tiles 0..3 + cs 0..1 prefetched in A0
                            if tt >= 2:
                                issue_cs_dma(1, tt)
                        xt = xts[(b, tt)]
                        cs, sn = css[(b, tt)]

                        def emit_v(tt=tt, xt=xt):
                            for t4 in range(4):
                                ps_t = ppv.tile([128, 512], f32, tag="ppv", name="ppv")
                                for e in range(8):
                                    nc.tensor.matmul(
                                        ps_t[:],
                                        xt[:, e, t4 * 128:(t4 + 1) * 128],
                                        wv[e][:],
                                        start=(e == 0), stop=(e == 7))
                                if tt == 3 and t4 % 2 == 1:
                                    # split the last tile's evacs across ACT
                                    # and DVE so neither engine's backlog
                                    # delays the attention phase's first
                                    # exp (ACT) / mask-add (DVE)
                                    nc.vector.tensor_copy(
                                        vv_chunks[tt * 4 + t4][:], ps_t[:])
                                else:
                                    nc.scalar.copy(vv_chunks[tt * 4 + t4][:], ps_t[:])

                        # v first (ACT evacuation, no cos/sin dependency)
                        # except on the last token tile, where qk-first ends
                        # the P phase with a short ACT tail instead of a long
                        # RoPE DVE tail.
                        if tt < 3:
                            emit_v()
                        for w_t, dstT in ((wq_t, qT_t), (wk_t, kT_t)):
                            for i, j, fo in ((0, 2, 0), (1, 3, 1)):
                                ps2 = []
                                for dc in (i, j):
                                    ps_t = pp.tile([128, 512], f32, tag="pp", name="pp")
                                    for e in range(8):
                                        nc.tensor.matmul(
                                            ps_t[:],
                                            w_t[:, e, dc * 128:(dc + 1) * 128],
                                            xt[:, e],
                                            start=(e == 0), stop=(e == 7))
                                    ps2.append(ps_t)
                                pi, pj = ps2
                                c_, s_ = cs[:, fo], sn[:, fo]
                                t0 = tp.tile([128, 512], f32, tag="rt", name="rt")
                                t1 = tp.tile([128, 512], f32, tag="rt", name="rt")
                                nc.vector.tensor_mul(t0[:], pi[:], c_)
                                nc.vector.tensor_mul(t1[:], pj[:], s_)
                                nc.vector.tensor_sub(
                                    dstT[i][:, s0:s0 + 512], t0[:], t1[:])
                                t2 = tp.tile([128, 512], f32, tag="rt", name="rt")
                                t3 = tp.tile([128, 512], f32, tag="rt", name="rt")
                                nc.vector.tensor_mul(t2[:], pi[:], s_)
                                nc.vector.tensor_mul(t3[:], pj[:], c_)
                                nc.vector.tensor_add(
                                    dstT[j][:, s0:s0 + 512], t2[:], t3[:])
                        if tt == 3:
                            emit_v()

                # ----- attention + o_proj phase (256-wide q tiles) -----
                with ExitStack() as actx:
                    ep = actx.enter_context(tc.tile_pool(name="ep", bufs=6))
                    atp = actx.enter_context(tc.tile_pool(name="atp", bufs=1))
                    ivp = actx.enter_context(tc.tile_pool(name="ivp", bufs=2))
                    obp = actx.enter_context(tc.tile_pool(name="obp", bufs=2))
                    ssp = actx.enter_context(tc.tile_pool(name="ssp", bufs=2))
                    # PSUM: matmul start=True zeroes the whole 2KB bank (the
                    # "zero region"), so every accumulator needs its own
                    # bank: 4 attn + 2 score + 2 shared o_proj/rowsum = 8.
                    scp = actx.enter_context(
                        tc.tile_pool(name="scp", bufs=2, space="PSUM"))
                    app = actx.enter_context(
                        tc.tile_pool(name="app", bufs=1, space="PSUM"))
                    opp = actx.enter_context(
                        tc.tile_pool(name="opp", bufs=2, space="PSUM"))

                    def emit_oproj(m):
                        # 1/rowsum is folded into the psum evacuation as a
                        # per-partition (per-token) scale. On the very last
                        # tile, evacuate+store in 256-wide pieces to shorten
                        # the serial tail after the final matmul.
                        q0 = m * QT
                        last = (b == 1 and m == NQT - 1)
                        for t4 in range(2):
                            ob = obp.tile([128, E], f32, tag="ob", name="ob")
                            r0 = tok0 + q0 + t4 * 128
                            for et in range(2):
                                op_ps = opp.tile([128, 512], f32, tag="op", name="op")
                                for dc in range(4):
                                    nc.tensor.matmul(
                                        op_ps[:],
                                        at_sb[m % 2][dc][:, t4 * 128:(t4 + 1) * 128],
                                        wo[dc][:, et * 512:(et + 1) * 512],
                                        start=(dc == 0), stop=(dc == 3))
                                nparts = 2 if (last and t4 == 1) else 1
                                w = 512 // nparts
                                for h in range(nparts):
                                    sl = slice(et * 512 + h * w, et * 512 + (h + 1) * w)
                                    if (et + h) % 2 == 0:
                                        nc.vector.tensor_scalar_mul(
                                            ob[:, sl], op_ps[:, h * w:(h + 1) * w],
                                            inv_sb[m % 2][:, t4:t4 + 1])
                                    else:
                                        nc.scalar.activation(
                                            ob[:, sl], op_ps[:, h * w:(h + 1) * w],
                                            AF.Copy,
                                            scale=inv_sb[m % 2][:, t4:t4 + 1])
                                    nc.sync.dma_start(out_d[r0:r0 + 128, sl],
                                                      ob[:, sl])

                    at_sb = {0: None, 1: None}
                    inv_sb = {0: None, 1: None}
                    for m in range(NQT):
                        q0 = m * QT
                        # off-diagonal 256-wide k-chunk ops; on the diagonal,
                        # chunk 2m runs full-width with an additive mask and
                        # chunk 2m+1 runs only its live 128-q half (its other
                        # half is fully above the diagonal).
                        ops = [(c, 0, QT, None) for c in range(2 * m)]
                        ops += [(2 * m, 0, QT, mk256), (2 * m + 1, 128, 128, mk128)]
                        nops = len(ops)
                        attn_ps = [app.tile([128, QT], f32, tag=f"attn{d}",
                                            name=f"attn{d}") for d in range(4)]
                        S = ssp.tile([128, QT], bf16, tag="S", name="S")

                        def emit_pv(exs, kc, qlo, qw, oi, nops=nops,
                                    attn_ps=attn_ps):
                            for dc in range(4):
                                nc.tensor.matmul(
                                    attn_ps[dc][:, qlo:qlo + qw],
                                    vv_chunks[kc][:, dc * 128:(dc + 1) * 128],
                                    exs,
                                    start=(oi == 0), stop=(oi == nops - 1))

                        pending = []
                        for oi, (kc, qlo, qw, mask) in enumerate(ops):
                            sc_t = scp.tile([128, QT], f32, tag="sc", name="sc")
                            sc_ps = sc_t[:, :qw]
                            for dc in range(4):
                                nc.tensor.matmul(
                                    sc_ps,
                                    kT_t[dc][:, kc * 128:(kc + 1) * 128],
                                    qT_t[dc][:, q0 + qlo:q0 + qlo + qw],
                                    start=(dc == 0), stop=(dc == 3))
                            if mask is not None:
                                nc.vector.tensor_add(sc_ps, sc_ps, mask)
                            ex = ep.tile([128, QT], bf16, tag="ex", name="ex")
                            exs = ex[:, :qw]
                            nc.scalar.activation(exs, sc_ps, AF.Exp, scale=SCALE)
                            if oi == 0:
                                nc.vector.tensor_copy(S[:], exs)
                            else:
                                nc.vector.tensor_add(S[:, qlo:qlo + qw],
                                                     S[:, qlo:qlo + qw], exs)
                            pending.append((exs, kc, qlo, qw, oi))
                            if len(pending) > 3:
                                emit_pv(*pending.pop(0))
                        for args in pending:
                            emit_pv(*args)
                        # transposed rowsum: rsT[q_local, t4] = sum_k S[k, q]
                        # via two 1-column matmuls (S halves as stationary),
                        # sharing one opp-pool bank (2nd accumulates into the
                        # bank zeroed by the 1st's start).
                        rs_full = opp.tile([128, 512], f32, tag="op", name="op")
                        nc.tensor.matmul(rs_full[:, 0:1], S[:, 0:128],
                                         onescol[:], start=True, stop=False)
                        nc.tensor.matmul(rs_full[:, 1:2], S[:, 128:256],
                                         onescol[:], start=False, stop=True)
                        inv2 = ivp.tile([128, 2], f32, tag="inv", name="inv")
                        nc.vector.reciprocal(inv2[:], rs_full[:, 0:2])
                        inv_sb[m % 2] = inv2
                        at_sb[m % 2] = [
                            atp.tile([128, QT], bf16, tag=f"at{m % 2}_{dc}",
                                     name=f"at{m % 2}_{dc}")
                            for dc in range(4)]
                        # balance the tile-tail psum evacuations across both
                        # ACT and DVE: on small early tiles either engine
                        # alone becomes the critical path
                        for dc in range(4):
                            if dc % 2 == 1:
                                nc.scalar.copy(
                                    at_sb[m % 2][dc][:], attn_ps[dc][:])
                            else:
                                nc.vector.tensor_copy(
                                    at_sb[m % 2][dc][:], attn_ps[dc][:])
                        if m > 0:
                            emit_oproj(m - 1)
                        if b == 0:
                            # prefetch batch-1 inputs while the PE is busy:
                            # HAM drops to 4/8 if it ever idles at the
                            # batch transition.
                            if 2 <= m <= 5:
                                issue_x_dma(1, m - 2)
                            if m == 6:
                                issue_cs_dma(1, 0)
                            if m == 7:
                                issue_cs_dma(1, 1)
                    emit_oproj(NQT - 1)
    nc.compile()
    return nc


def _host_tables():
    inv_freq = 1.0 / (ROPE_BASE ** (np.arange(0, D, 2, dtype=np.float64) / D))
    ang = np.arange(T, dtype=np.float64)[:, None] * inv_freq[None, :]  # [T, D/2]
    cosdt = np.ascontiguousarray(np.cos(ang).T.astype(np.float32))     # [D/2, T]
    sindt = np.ascontiguousarray(np.sin(ang).T.astype(np.float32))
    kk = np.arange(128)[:, None]
    masks = np.empty((128, 384), dtype=np.float32)
    masks[:, 0:QT] = np.where(kk <= np.arange(QT)[None, :], 0.0, NEG)
    masks[:, QT:384] = np.where(kk <= np.arange(128)[None, :], 0.0, NEG)
    return cosdt, sindt, masks


def kernel(x, Wq, Wk, Wv, Wo):
    global LAST_RESULTS
    import ml_dtypes
    from concourse import bass_utils

    if "nc" not in _CACHE:
        _CACHE["nc"] = _build()
    nc = _CACHE["nc"]

    bf16 = ml_dtypes.bfloat16
    x = np.asarray(x, dtype=np.float32)
    Wq = np.asarray(Wq, dtype=np.float32)
    Wk = np.asarray(Wk, dtype=np.float32)
    Wv = np.asarray(Wv, dtype=np.float32)
    Wo = np.asarray(Wo, dtype=np.float32)

    xT = np.ascontiguousarray(x.reshape(NTOK, E).T).astype(bf16)  # [E, NTOK]
    cosdt, sindt, masks = _host_tables()

    in_maps = []
    for h in range(H):
        in_maps.append({
            "xT": xT,
            "wqT": np.ascontiguousarray(Wq[h * D:(h + 1) * D, :].T).astype(bf16),
            "wkT": np.ascontiguousarray(Wk[h * D:(h + 1) * D, :].T).astype(bf16),
            "wvT": np.ascontiguousarray(Wv[h * D:(h + 1) * D, :].T).astype(bf16),
            "woT": np.ascontiguousarray(Wo[:, h * D:(h + 1) * D].T).astype(bf16),
            "cosdt": cosdt,
            "sindt": sindt,
            "masks": masks,
        })

    kwargs = {}
    if PROFILE:
        import sys
        import types
        import trn_agent_boot.trn_boot as _tb
        hook = _tb._ntff_profile_via_ctypes("/opt/axon/libaxon_pjrt.so")
        mod = types.ModuleType("antenv.axon_hooks")
        mod.get_axon_ntff_profile_hook = lambda: hook
        mod.set_axon_ntff_profile_hook = lambda h_: None
        sys.modules["antenv.axon_hooks"] = mod
        bass_utils.upload_artifacts = lambda tmpdir: tmpdir
        kwargs = dict(trace=True, trace_cores=[0])

    res = bass_utils.run_bass_kernel_spmd(
        nc, in_maps, core_ids=list(range(H)), **kwargs)
    LAST_RESULTS = res

    out = res.results[0]["out"].astype(np.float32).copy()
    for h in range(1, H):
        out += res.results[h]["out"]
    return out.reshape(B, T, E)


# revision 66
# speedup vs baseline: 1.0072x; 1.0072x over previous
"""Trainium2 Bass kernel: multi-head attention (B=2, T=2048, E=1024, H=8, D=512),
bias-free QKV/O projections + RoPE + causal softmax.

Sharding: head-parallel across 8 NeuronCores. Core h computes head h fully;
host sums the 8 partial o_proj outputs (the all-reduce after o_proj).

v2 layout (vs 452us baseline):
  - x / Wq / Wk / Wv / qT / kT / Wo / at_sb in bf16 (same 1 cycle/row on PE,
    half DMA + SBUF); v / probs / rowsum stay f32r for accuracy.
  - attention at 256-wide q tiles (2m+2 causal k-chunks of 128) instead of
    512-wide (4n+4): less masked-diagonal waste on the PE.
  - rowsum via DVE accumulation of exp tiles (S += ex) + ONE 256-row
    ones-matmul per q tile instead of a 512-row matmul per chunk.
  - PE never idles: batch-1 x / cos / sin prefetched during batch-0
    attention (the HAM duty-cycle drops 8/8 -> 4/8 on any PE idle gap and
    costs ~14us to recover); warmup matmuls bridge the startup DMA.
"""
from contextlib import ExitStack

import numpy as np

B, T, E, H, D = 2, 2048, 1024, 8, 512
NTOK = B * T
SCALE = float(1.0 / np.sqrt(D))
NEG = -1.0e30
ROPE_BASE = 10000.0
QT = 256          # attention q-tile width
NQT = T // QT     # 8 q tiles per batch
WARM = 12         # warmup matmuls (512 rows each) bridging startup DMA

PROFILE = False          # set True (e.g. from test.py) to trace core 0
LAST_RESULTS = None      # BassKernelResults of the last run when PROFILE

_CACHE = {}


def _build():
    import concourse.tile as tile
    from concourse import bacc, mybir

    f32 = mybir.dt.float32
    f32r = mybir.dt.float32r
    bf16 = mybir.dt.bfloat16
    AF = mybir.ActivationFunctionType

    nc = bacc.Bacc("TRN2", target_bir_lowering=False, debug=False,
                   enable_asserts=False, num_devices=8)
    xT_d = nc.dram_tensor("xT", [E, NTOK], bf16, kind="ExternalInput").ap()
    wqT_d = nc.dram_tensor("wqT", [E, D], bf16, kind="ExternalInput").ap()
    wkT_d = nc.dram_tensor("wkT", [E, D], bf16, kind="ExternalInput").ap()
    wvT_d = nc.dram_tensor("wvT", [E, D], bf16, kind="ExternalInput").ap()
    woT_d = nc.dram_tensor("woT", [D, E], f32r, kind="ExternalInput").ap()
    cos_d = nc.dram_tensor("cosdt", [D // 2, T], f32, kind="ExternalInput").ap()
    sin_d = nc.dram_tensor("sindt", [D // 2, T], f32, kind="ExternalInput").ap()
    msk_d = nc.dram_tensor("masks", [128, 384], f32, kind="ExternalInput").ap()
    out_d = nc.dram_tensor("out", [NTOK, E], f32, kind="ExternalOutput").ap()

    xT_r = xT_d.rearrange("(eo p) t -> p eo t", p=128)     # [128, 8, 4096]
    cos_r = cos_d.rearrange("(fo p) t -> p fo t", p=128)   # [128, 2, 2048]
    sin_r = sin_d.rearrange("(fo p) t -> p fo t", p=128)
    wq_r = wqT_d.rearrange("(eo p) d -> p eo d", p=128)
    wk_r = wkT_d.rearrange("(eo p) d -> p eo d", p=128)
    wv_r = wvT_d.rearrange("(eo p) d -> p eo d", p=128)
    wo_r = woT_d.rearrange("(do p) e -> p do e", p=128)

    with tile.TileContext(nc) as tc, ExitStack() as top:
        wp = top.enter_context(tc.tile_pool(name="wp", bufs=1))
        wq_t = wp.tile([128, 8, D], bf16, tag="wq", name="wq")
        wk_t = wp.tile([128, 8, D], bf16, tag="wk", name="wk")
        wv_t = wp.tile([128, 8, D], bf16, tag="wv", name="wv")
        wv = [wv_t[:, e] for e in range(8)]
        wo_t = wp.tile([128, 4, E], f32r, tag="wo", name="wo")
        wo = [wo_t[:, d] for d in range(4)]
        mks = wp.tile([128, 384], f32, tag="mks", name="mks")
        mk256 = mks[:, 0:QT]     # diag chunk 2m mask over a full 256-q tile
        mk128 = mks[:, QT:384]   # diag chunk 2m+1 mask over its live 128-q half
        # bf16: the rowsum-transpose matmuls have a 1-wide moving dim (fp32r
        # forbids that) and bf16 avoids a PE mode switch mid-stream.
        onescol = wp.tile([128, 1], bf16, tag="onescol", name="onescol")

        # x tiles: one rolling pool across both batches so batch-1 tiles can
        # be prefetched (DMA'd) while batch-0 attention runs.
        xp = top.enter_context(tc.tile_pool(name="xp", bufs=4))
        csp = top.enter_context(tc.tile_pool(name="csp", bufs=2))

        xts = {}   # (b, tt) -> tile
        css = {}   # (b, tt) -> (cs, sn)

        def issue_x_dma(b, tt):
            t = xp.tile([128, 8, 512], bf16, tag="xt", name="xt")
            g0 = b * T + tt * 512
            nc.sync.dma_start(t[:], xT_r[:, :, g0:g0 + 512])
            xts[(b, tt)] = t

        def issue_cs_dma(b, tt):
            s0 = tt * 512
            cs = csp.tile([128, 2, 512], f32, tag="cs", name="cs")
            sn = csp.tile([128, 2, 512], f32, tag="sn", name="sn")
            nc.sync.dma_start(cs[:], cos_r[:, :, s0:s0 + 512])
            nc.sync.dma_start(sn[:], sin_r[:, :, s0:s0 + 512])
            css[(b, tt)] = (cs, sn)

        for b in range(B):
            tok0 = b * T
            with ExitStack() as bctx:
                qkv = bctx.enter_context(tc.tile_pool(name="qkv", bufs=1))
                qT_t = [qkv.tile([128, T], bf16, tag=f"qT{d}", name=f"qT{d}") for d in range(4)]
                kT_t = [qkv.tile([128, T], bf16, tag=f"kT{d}", name=f"kT{d}") for d in range(4)]
                vv_chunks = [qkv.tile([128, D], bf16, tag=f"v{t}", name=f"v{t}")
                             for t in range(16)]

                # ----- projection phase: qT/kT (RoPE'd) and v -----
                with ExitStack() as pctx:
                    tp = pctx.enter_context(tc.tile_pool(name="tp", bufs=4))
                    pp = pctx.enter_context(
                        tc.tile_pool(name="pp", bufs=6, space="PSUM"))
                    ppv = pctx.enter_context(
                        tc.tile_pool(name="ppv", bufs=2, space="PSUM"))

                    if b == 0:
                        # warmup: PE-busy filler while startup DMA streams in;
                        # lifts the HAM clock gate to 8/8 and produces the
                        # `ones` tile (WARM accumulated ones.T@ones passes).
                        warmp = pctx.enter_context(
                            tc.tile_pool(name="warmp", bufs=1))
                        onef = warmp.tile([128, 128], f32, tag="onef", name="onef")
                        nc.vector.memset(onef[:], 1.0)
                        ones0 = warmp.tile([128, 128], f32r, tag="ones0", name="ones0")
                        nc.vector.tensor_copy(ones0[:], onef[:])
                        nc.vector.memset(onescol[:], 1.0)
                        wsf = warmp.tile([128, 512], f32, tag="wsf", name="wsf")
                        nc.vector.memset(wsf[:], 1.0)
                        wsrc = warmp.tile([128, 512], f32r, tag="wsrc", name="wsrc")
                        nc.vector.tensor_copy(wsrc[:], wsf[:])
                        warm_ps = pp.tile([128, 512], f32, tag="pp", name="pp")
                        for w in range(WARM):
                            nc.tensor.matmul(warm_ps[:], ones0[:], wsrc[:],
                                             start=(w == 0), stop=(w == WARM - 1))
                        # touch Exp so its ACT table set loads during the
                        # DMA-bound startup instead of at the first score tile
                        expre = warmp.tile([128, 1], f32, tag="expre", name="expre")
                        nc.scalar.activation(expre[:], warm_ps[:, :1], AF.Exp,
                                             scale=0.001)
                        nc.vector.tensor_copy(expre[:], expre[:])

                    for tt in range(4):
                        s0 = tt * 512
                        if b == 0:
                            if tt == 0:
                                # need-ordered startup loads, halved so the
                                # first v matmuls (xt e0-3 + wv e0-3) start
                                # as early as possible.
                                t = xp.tile([128, 8, 512], bf16, tag="xt", name="xt")
                                nc.sync.dma_start(t[:, 0:4], xT_r[:, 0:4, 0:512])
                                nc.sync.dma_start(wv_t[:, 0:4], wv_r[:, 0:4])
                                nc.sync.dma_start(wq_t[:, 0:4], wq_r[:, 0:4])
                                nc.sync.dma_start(t[:, 4:8], xT_r[:, 4:8, 0:512])
                                nc.sync.dma_start(wv_t[:, 4:8], wv_r[:, 4:8])
                                nc.sync.dma_start(wq_t[:, 4:8], wq_r[:, 4:8])
                                xts[(0, 0)] = t
                                issue_cs_dma(0, 0)
                                nc.sync.dma_start(mks[:], msk_d)
                                nc.sync.dma_start(wk_t[:], wk_r)
                            else:
                                issue_x_dma(0, tt)
                                issue_cs_dma(0, tt)
                                if tt == 1:
                                    nc.sync.dma_start(wo_t[:], wo_r)
                        else:
                            # batch 1: tiles 0..3 + cs 0..1 prefetched in A0
                            if tt >= 2:
                                issue_cs_dma(1, tt)
                        xt = xts[(b, tt)]
                        cs, sn = css[(b, tt)]

                        def emit_v(tt=tt, xt=xt):
                            for t4 in range(4):
                                ps_t = ppv.tile([128, 512], f32, tag="ppv", name="ppv")
                                for e in range(8):
                                    nc.tensor.matmul(
                                        ps_t[:],
                                        xt[:, e, t4 * 128:(t4 + 1) * 128],
                                        wv[e][:],
                                        start=(e == 0), stop=(e == 7))
                                if tt == 3 and t4 % 2 == 1:
                                    # split the last tile's evacs across ACT
                                    # and DVE so neither engine's backlog
                                    # delays the attention phase's first
                                    # exp (ACT) / mask-add (DVE)
                                    nc.vector.tensor_copy(
                                        vv_chunks[tt * 4 + t4][:], ps_t[:])
                                else:
                                    nc.scalar.copy(vv_chunks[tt * 4 + t4][:], ps_t[:])

                        # v first (ACT evacuation, no cos/sin dependency)
                        # except on the last token tile, where qk-first ends
                        # the P phase with a short ACT tail instead of a long
                        # RoPE DVE tail.
                        if tt < 3:
                            emit_v()
                        for w_t, dstT in ((wq_t, qT_t), (wk_t, kT_t)):
                            for i, j, fo in ((0, 2, 0), (1, 3, 1)):
                                ps2 = []
                                for dc in (i, j):
                                    ps_t = pp.tile([128, 512], f32, tag="pp", name="pp")
                                    for e in range(8):
                                        nc.tensor.matmul(
                                            ps_t[:],
                                            w_t[:, e, dc * 128:(dc + 1) * 128],
                                            xt[:, e],
                                            start=(e == 0), stop=(e == 7))
                                    ps2.append(ps_t)
                                pi, pj = ps2
                                c_, s_ = cs[:, fo], sn[:, fo]
                                t0 = tp.tile([128, 512], f32, tag="rt", name="rt")
                                t1 = tp.tile([128, 512], f32, tag="rt", name="rt")
                                nc.vector.tensor_mul(t0[:], pi[:], c_)
                                nc.vector.tensor_mul(t1[:], pj[:], s_)
                                nc.vector.tensor_sub(
                                    dstT[i][:, s0:s0 + 512], t0[:], t1[:])
                                t2 = tp.tile([128, 512], f32, tag="rt", name="rt")
                                t3 = tp.tile([128, 512], f32, tag="rt", name="rt")
                                nc.vector.tensor_mul(t2[:], pi[:], s_)
                                nc.vector.tensor_mul(t3[:], pj[:], c_)
                                nc.vector.tensor_add(
                                    dstT[j][:, s0:s0 + 512], t2[:], t3[:])
                        if tt == 3:
                            emit_v()

                # ----- attention + o_proj phase (256-wide q tiles) -----
                with ExitStack() as actx:
                    ep = actx.enter_context(tc.tile_pool(name="ep", bufs=6))
                    atp = actx.enter_context(tc.tile_pool(name="atp", bufs=1))
                    ivp = actx.enter_context(tc.tile_pool(name="ivp", bufs=2))
                    obp = actx.enter_context(tc.tile_pool(name="obp", bufs=2))
                    ssp = actx.enter_context(tc.tile_pool(name="ssp", bufs=2))
                    # PSUM: matmul start=True zeroes the whole 2KB bank (the
                    # "zero region"), so every accumulator needs its own
                    # bank: 4 attn + 2 score + 2 shared o_proj/rowsum = 8.
                    # attn accumulators pair-packed 2-per-bank (first of a
                    # pair start=True zeroes the bank, second accumulates
                    # into zeroed space): 2 + scp 3 + opp 2 = 7 banks.
                    scp = actx.enter_context(
                        tc.tile_pool(name="scp", bufs=3, space="PSUM"))
                    app = actx.enter_context(
                        tc.tile_pool(name="app", bufs=1, space="PSUM"))
                    opp = actx.enter_context(
                        tc.tile_pool(name="opp", bufs=2, space="PSUM"))

                    def emit_oproj(m):
                        # 1/rowsum is folded into the psum evacuation as a
                        # per-partition (per-token) scale. On the very last
                        # tile, evacuate+store in 256-wide pieces to shorten
                        # the serial tail after the final matmul.
                        q0 = m * QT
                        last = (b == 1 and m == NQT - 1)
                        for t4 in range(2):
                            ob = obp.tile([128, E], f32, tag="ob", name="ob")
                            r0 = tok0 + q0 + t4 * 128
                            for et in range(2):
                                op_ps = opp.tile([128, 512], f32, tag="op", name="op")
                                for dc in range(4):
                                    a0 = dc * QT + t4 * 128
                                    nc.tensor.matmul(
                                        op_ps[:],
                                        at_sb[m % 2][:, a0:a0 + 128],
                                        wo[dc][:, et * 512:(et + 1) * 512],
                                        start=(dc == 0), stop=(dc == 3))
                                nparts = 2 if (last and t4 == 1) else 1
                                w = 512 // nparts
                                for h in range(nparts):
                                    sl = slice(et * 512 + h * w, et * 512 + (h + 1) * w)
                                    if last and h == 1:
                                        nc.scalar.activation(
                                            ob[:, sl], op_ps[:, h * w:(h + 1) * w],
                                            AF.Copy,
                                            scale=inv_sb[m % 2][:, t4:t4 + 1])
                                    else:
                                        nc.vector.tensor_scalar_mul(
                                            ob[:, sl], op_ps[:, h * w:(h + 1) * w],
                                            inv_sb[m % 2][:, t4:t4 + 1])
                                    nc.sync.dma_start(out_d[r0:r0 + 128, sl],
                                                      ob[:, sl])

                    at_sb = {0: None, 1: None}
                    inv_sb = {0: None, 1: None}
                    for m in range(NQT):
                        q0 = m * QT
                        # off-diagonal 256-wide k-chunk ops; on the diagonal,
                        # chunk 2m runs full-width with an additive mask and
                        # chunk 2m+1 runs only its live 128-q half (its other
                        # half is fully above the diagonal).
                        ops = [(c, 0, QT, None) for c in range(2 * m)]
                        ops += [(2 * m, 0, QT, mk256), (2 * m + 1, 128, 128, mk128)]
                        nops = len(ops)
                        attnAB = [app.tile([128, 512], f32, tag=f"attnb{i}",
                                           name=f"attnb{i}") for i in range(2)]
                        attn_ps = [attnAB[d // 2][:, (d % 2) * QT:(d % 2 + 1) * QT]
                                   for d in range(4)]
                        S = ssp.tile([128, QT], bf16, tag="S", name="S")

                        def emit_pv(exs, kc, qlo, qw, oi, nops=nops,
                                    attn_ps=attn_ps):
                            for dc in range(4):
                                nc.tensor.matmul(
                                    attn_ps[dc][:, qlo:qlo + qw],
                                    vv_chunks[kc][:, dc * 128:(dc + 1) * 128],
                                    exs,
                                    start=(oi == 0 and dc % 2 == 0),
                                    stop=(oi == nops - 1 and dc % 2 == 1))

                        pending = []
                        for oi, (kc, qlo, qw, mask) in enumerate(ops):
                            sc_t = scp.tile([128, QT], f32, tag="sc", name="sc")
                            sc_ps = sc_t[:, :qw]
                            for dc in range(4):
                                nc.tensor.matmul(
                                    sc_ps,
                                    kT_t[dc][:, kc * 128:(kc + 1) * 128],
                                    qT_t[dc][:, q0 + qlo:q0 + qlo + qw],
                                    start=(dc == 0), stop=(dc == 3))
                            if mask is not None:
                                nc.vector.tensor_add(sc_ps, sc_ps, mask)
                            ex = ep.tile([128, QT], bf16, tag="ex", name="ex")
                            exs = ex[:, :qw]
                            nc.scalar.activation(exs, sc_ps, AF.Exp, scale=SCALE)
                            if oi == 0:
                                nc.vector.tensor_copy(S[:], exs)
                            else:
                                nc.vector.tensor_add(S[:, qlo:qlo + qw],
                                                     S[:, qlo:qlo + qw], exs)
                            pending.append((exs, kc, qlo, qw, oi))
                            if len(pending) > 3:
                                emit_pv(*pending.pop(0))
                        for args in pending:
                            emit_pv(*args)
                        # transposed rowsum: rsT[q_local, t4] = sum_k S[k, q]
                        # via two 1-column matmuls (S halves as stationary),
                        # sharing one opp-pool bank (2nd accumulates into the
                        # bank zeroed by the 1st's start).
                        rs_full = opp.tile([128, 512], f32, tag="op", name="op")
                        nc.tensor.matmul(rs_full[:, 0:1], S[:, 0:128],
                                         onescol[:], start=True, stop=False)
                        nc.tensor.matmul(rs_full[:, 1:2], S[:, 128:256],
                                         onescol[:], start=False, stop=True)
                        inv2 = ivp.tile([128, 2], f32, tag="inv", name="inv")
                        nc.vector.reciprocal(inv2[:], rs_full[:, 0:2])
                        inv_sb[m % 2] = inv2
                        # evacuate both packed banks with two wide f32r
                        # copies (fp32-rate, no bf16 CAST). They stay on DVE:
                        # ACT work queued here delays the next tile's exps
                        # (FIFO), which stalls PV. Only the final tile (no
                        # exps after it) puts one half on ACT to pace the
                        # immediately-following o_proj.
                        at_t = atp.tile([128, 4 * QT], f32r,
                                        tag=f"at{m % 2}", name=f"at{m % 2}")
                        at_sb[m % 2] = at_t
                        for half in range(2):
                            sl = slice(half * 512, (half + 1) * 512)
                            if b == 1 and m == NQT - 1 and half == 1:
                                nc.scalar.copy(at_t[:, sl], attnAB[half][:])
                            else:
                                nc.vector.tensor_copy(at_t[:, sl], attnAB[half][:])
                        if m > 0:
                            emit_oproj(m - 1)
                        if b == 0:
                            # prefetch batch-1 inputs while the PE is busy:
                            # HAM drops to 4/8 if it ever idles at the
                            # batch transition.
                            if 2 <= m <= 5:
                                issue_x_dma(1, m - 2)
                            if m == 6:
                                issue_cs_dma(1, 0)
                            if m == 7:
                                issue_cs_dma(1, 1)
                    emit_oproj(NQT - 1)
    nc.compile()
    return nc


def _host_tables():
    inv_freq = 1.0 / (ROPE_BASE ** (np.arange(0, D, 2, dtype=np.float64) / D))
    ang = np.arange(T, dtype=np.float64)[:, None] * inv_freq[None, :]  # [T, D/2]
    cosdt = np.ascontiguousarray(np.cos(ang).T.astype(np.float32))     # [D/2, T]
    sindt = np.ascontiguousarray(np.sin(ang).T.astype(np.float32))
    kk = np.arange(128)[:, None]
    masks = np.empty((128, 384), dtype=np.float32)
    masks[:, 0:QT] = np.where(kk <= np.arange(QT)[None, :], 0.0, NEG)
    masks[:, QT:384] = np.where(kk <= np.arange(128)[None, :], 0.0, NEG)
    return cosdt, sindt, masks


def kernel(x, Wq, Wk, Wv, Wo):
    global LAST_RESULTS
    import ml_dtypes
    from concourse import bass_utils

    if "nc" not in _CACHE:
        _CACHE["nc"] = _build()
    nc = _CACHE["nc"]

    bf16 = ml_dtypes.bfloat16
    x = np.asarray(x, dtype=np.float32)
    Wq = np.asarray(Wq, dtype=np.float32)
    Wk = np.asarray(Wk, dtype=np.float32)
    Wv = np.asarray(Wv, dtype=np.float32)
    Wo = np.asarray(Wo, dtype=np.float32)

    xT = np.ascontiguousarray(x.reshape(NTOK, E).T).astype(bf16)  # [E, NTOK]
    cosdt, sindt, masks = _host_tables()

    in_maps = []
    for h in range(H):
        in_maps.append({
            "xT": xT,
            "wqT": np.ascontiguousarray(Wq[h * D:(h + 1) * D, :].T).astype(bf16),
            "wkT": np.ascontiguousarray(Wk[h * D:(h + 1) * D, :].T).astype(bf16),
            "wvT": np.ascontiguousarray(Wv[h * D:(h + 1) * D, :].T).astype(bf16),
            "woT": np.ascontiguousarray(Wo[:, h * D:(h + 1) * D].T),
            "cosdt": cosdt,
            "sindt": sindt,
            "masks": masks,
        })

    kwargs = {}
    if PROFILE:
        import sys
        import types
        import trn_agent_boot.trn_boot as _tb
        hook = _tb._ntff_profile_via_ctypes("/opt/axon/libaxon_pjrt.so")
        mod = types.ModuleType("antenv.axon_hooks")
        mod.get_axon_ntff_profile_hook = lambda: hook
        mod.set_axon_ntff_profile_hook = lambda h_: None
        sys.modules["antenv.axon_hooks"] = mod
        bass_utils.upload_artifacts = lambda tmpdir: tmpdir
        kwargs = dict(trace=True, trace_cores=[0])

    res = bass_utils.run_bass_kernel_spmd(
        nc, in_maps, core_ids=list(range(H)), **kwargs)
    LAST_RESULTS = res

    out = res.results[0]["out"].astype(np.float32).copy()
    for h in range(1, H):
        out += res.results[h]["out"]
    return out.reshape(B, T, E)


# revision 74
# speedup vs baseline: 1.0191x; 1.0118x over previous
"""Trainium2 Bass kernel: multi-head attention (B=2, T=2048, E=1024, H=8, D=512),
bias-free QKV/O projections + RoPE + causal softmax.

Sharding: head-parallel across 8 NeuronCores. Core h computes head h fully;
host sums the 8 partial o_proj outputs (the all-reduce after o_proj).

v2 layout (vs 452us baseline):
  - x / Wq / Wk / Wv / qT / kT / Wo / at_sb in bf16 (same 1 cycle/row on PE,
    half DMA + SBUF); v / probs / rowsum stay f32r for accuracy.
  - attention at 256-wide q tiles (2m+2 causal k-chunks of 128) instead of
    512-wide (4n+4): less masked-diagonal waste on the PE.
  - rowsum via DVE accumulation of exp tiles (S += ex) + ONE 256-row
    ones-matmul per q tile instead of a 512-row matmul per chunk.
  - PE never idles: batch-1 x / cos / sin prefetched during batch-0
    attention (the HAM duty-cycle drops 8/8 -> 4/8 on any PE idle gap and
    costs ~14us to recover); warmup matmuls bridge the startup DMA.
"""
from contextlib import ExitStack

import numpy as np

B, T, E, H, D = 2, 2048, 1024, 8, 512
NTOK = B * T
SCALE = float(1.0 / np.sqrt(D))
NEG = -1.0e30
ROPE_BASE = 10000.0
QT = 256          # attention q-tile width
NQT = T // QT     # 8 q tiles per batch
WARM = 12         # warmup matmuls (512 rows each) bridging startup DMA

PROFILE = False          # set True (e.g. from test.py) to trace core 0
LAST_RESULTS = None      # BassKernelResults of the last run when PROFILE

_CACHE = {}


def _build():
    import concourse.tile as tile
    from concourse import bacc, mybir

    f32 = mybir.dt.float32
    f32r = mybir.dt.float32r
    bf16 = mybir.dt.bfloat16
    AF = mybir.ActivationFunctionType

    nc = bacc.Bacc("TRN2", target_bir_lowering=False, debug=False,
                   enable_asserts=False, num_devices=8)
    xT_d = nc.dram_tensor("xT", [E, NTOK], bf16, kind="ExternalInput").ap()
    wqT_d = nc.dram_tensor("wqT", [E, D], bf16, kind="ExternalInput").ap()
    wkT_d = nc.dram_tensor("wkT", [E, D], bf16, kind="ExternalInput").ap()
    wvT_d = nc.dram_tensor("wvT", [E, D], bf16, kind="ExternalInput").ap()
    woT_d = nc.dram_tensor("woT", [D, E], bf16, kind="ExternalInput").ap()
    cos_d = nc.dram_tensor("cosdt", [D // 2, T], f32, kind="ExternalInput").ap()
    sin_d = nc.dram_tensor("sindt", [D // 2, T], f32, kind="ExternalInput").ap()
    msk_d = nc.dram_tensor("masks", [128, 384], f32, kind="ExternalInput").ap()
    out_d = nc.dram_tensor("out", [NTOK, E], f32, kind="ExternalOutput").ap()

    xT_r = xT_d.rearrange("(eo p) t -> p eo t", p=128)     # [128, 8, 4096]
    cos_r = cos_d.rearrange("(fo p) t -> p fo t", p=128)   # [128, 2, 2048]
    sin_r = sin_d.rearrange("(fo p) t -> p fo t", p=128)
    wq_r = wqT_d.rearrange("(eo p) d -> p eo d", p=128)
    wk_r = wkT_d.rearrange("(eo p) d -> p eo d", p=128)
    wv_r = wvT_d.rearrange("(eo p) d -> p eo d", p=128)
    wo_r = woT_d.rearrange("(do p) e -> p do e", p=128)

    with tile.TileContext(nc) as tc, ExitStack() as top:
        wp = top.enter_context(tc.tile_pool(name="wp", bufs=1))
        wq_t = wp.tile([128, 8, D], bf16, tag="wq", name="wq")
        wk_t = wp.tile([128, 8, D], bf16, tag="wk", name="wk")
        wv_t = wp.tile([128, 8, D], bf16, tag="wv", name="wv")
        wv = [wv_t[:, e] for e in range(8)]
        wo_t = wp.tile([128, 4, E], bf16, tag="wo", name="wo")
        wo = [wo_t[:, d] for d in range(4)]
        mks = wp.tile([128, 384], f32, tag="mks", name="mks")
        mk256 = mks[:, 0:QT]     # diag chunk 2m mask over a full 256-q tile
        mk128 = mks[:, QT:384]   # diag chunk 2m+1 mask over its live 128-q half
        # bf16: the rowsum-transpose matmuls have a 1-wide moving dim (fp32r
        # forbids that) and bf16 avoids a PE mode switch mid-stream.
        onescol = wp.tile([128, 1], bf16, tag="onescol", name="onescol")

        # x tiles: one rolling pool across both batches so batch-1 tiles can
        # be prefetched (DMA'd) while batch-0 attention runs.
        xp = top.enter_context(tc.tile_pool(name="xp", bufs=4))
        csp = top.enter_context(tc.tile_pool(name="csp", bufs=2))

        xts = {}   # (b, tt) -> tile
        css = {}   # (b, tt) -> (cs, sn)

        def issue_x_dma(b, tt):
            t = xp.tile([128, 8, 512], bf16, tag="xt", name="xt")
            g0 = b * T + tt * 512
            nc.sync.dma_start(t[:], xT_r[:, :, g0:g0 + 512])
            xts[(b, tt)] = t

        def issue_cs_dma(b, tt):
            s0 = tt * 512
            cs = csp.tile([128, 2, 512], f32, tag="cs", name="cs")
            sn = csp.tile([128, 2, 512], f32, tag="sn", name="sn")
            nc.sync.dma_start(cs[:], cos_r[:, :, s0:s0 + 512])
            nc.sync.dma_start(sn[:], sin_r[:, :, s0:s0 + 512])
            css[(b, tt)] = (cs, sn)

        for b in range(B):
            tok0 = b * T
            with ExitStack() as bctx:
                qkv = bctx.enter_context(tc.tile_pool(name="qkv", bufs=1))
                qT_t = [qkv.tile([128, T], bf16, tag=f"qT{d}", name=f"qT{d}") for d in range(4)]
                kT_t = [qkv.tile([128, T], bf16, tag=f"kT{d}", name=f"kT{d}") for d in range(4)]
                vv_chunks = [qkv.tile([128, D], bf16, tag=f"v{t}", name=f"v{t}")
                             for t in range(16)]

                # ----- projection phase: qT/kT (RoPE'd) and v -----
                with ExitStack() as pctx:
                    tp = pctx.enter_context(tc.tile_pool(name="tp", bufs=4))
                    pp = pctx.enter_context(
                        tc.tile_pool(name="pp", bufs=6, space="PSUM"))
                    ppv = pctx.enter_context(
                        tc.tile_pool(name="ppv", bufs=2, space="PSUM"))

                    if b == 0:
                        # warmup: PE-busy filler while startup DMA streams in;
                        # lifts the HAM clock gate to 8/8 and produces the
                        # `ones` tile (WARM accumulated ones.T@ones passes).
                        warmp = pctx.enter_context(
                            tc.tile_pool(name="warmp", bufs=1))
                        onef = warmp.tile([128, 128], f32, tag="onef", name="onef")
                        nc.vector.memset(onef[:], 1.0)
                        ones0 = warmp.tile([128, 128], f32r, tag="ones0", name="ones0")
                        nc.vector.tensor_copy(ones0[:], onef[:])
                        nc.vector.memset(onescol[:], 1.0)
                        wsf = warmp.tile([128, 512], f32, tag="wsf", name="wsf")
                        nc.vector.memset(wsf[:], 1.0)
                        wsrc = warmp.tile([128, 512], f32r, tag="wsrc", name="wsrc")
                        nc.vector.tensor_copy(wsrc[:], wsf[:])
                        warm_ps = pp.tile([128, 512], f32, tag="pp", name="pp")
                        for w in range(WARM):
                            nc.tensor.matmul(warm_ps[:], ones0[:], wsrc[:],
                                             start=(w == 0), stop=(w == WARM - 1))
                        # touch Exp so its ACT table set loads during the
                        # DMA-bound startup instead of at the first score tile
                        expre = warmp.tile([128, 1], f32, tag="expre", name="expre")
                        nc.scalar.activation(expre[:], warm_ps[:, :1], AF.Exp,
                                             scale=0.001)
                        nc.vector.tensor_copy(expre[:], expre[:])

                    for tt in range(4):
                        s0 = tt * 512
                        if b == 0:
                            if tt == 0:
                                # need-ordered startup loads, halved so the
                                # first v matmuls (xt e0-3 + wv e0-3) start
                                # as early as possible.
                                t = xp.tile([128, 8, 512], bf16, tag="xt", name="xt")
                                nc.sync.dma_start(t[:, 0:4], xT_r[:, 0:4, 0:512])
                                nc.sync.dma_start(wv_t[:, 0:4], wv_r[:, 0:4])
                                nc.sync.dma_start(wq_t[:, 0:4], wq_r[:, 0:4])
                                nc.sync.dma_start(t[:, 4:8], xT_r[:, 4:8, 0:512])
                                nc.sync.dma_start(wv_t[:, 4:8], wv_r[:, 4:8])
                                nc.sync.dma_start(wq_t[:, 4:8], wq_r[:, 4:8])
                                xts[(0, 0)] = t
                                issue_cs_dma(0, 0)
                                nc.sync.dma_start(mks[:], msk_d)
                                nc.sync.dma_start(wk_t[:], wk_r)
                            else:
                                issue_x_dma(0, tt)
                                issue_cs_dma(0, tt)
                                if tt == 1:
                                    nc.sync.dma_start(wo_t[:], wo_r)
                        else:
                            # batch 1: tiles 0..3 + cs 0..1 prefetched in A0
                            if tt >= 2:
                                issue_cs_dma(1, tt)
                        xt = xts[(b, tt)]
                        cs, sn = css[(b, tt)]

                        def emit_v(tt=tt, xt=xt):
                            for t4 in range(4):
                                ps_t = ppv.tile([128, 512], f32, tag="ppv", name="ppv")
                                for e in range(8):
                                    nc.tensor.matmul(
                                        ps_t[:],
                                        xt[:, e, t4 * 128:(t4 + 1) * 128],
                                        wv[e][:],
                                        start=(e == 0), stop=(e == 7))
                                if tt == 3 and t4 % 2 == 1:
                                    # split the last tile's evacs across ACT
                                    # and DVE so neither engine's backlog
                                    # delays the attention phase's first
                                    # exp (ACT) / mask-add (DVE)
                                    nc.vector.tensor_copy(
                                        vv_chunks[tt * 4 + t4][:], ps_t[:])
                                else:
                                    nc.scalar.copy(vv_chunks[tt * 4 + t4][:], ps_t[:])

                        # v first (ACT evacuation, no cos/sin dependency)
                        # except on the last token tile, where qk-first ends
                        # the P phase with a short ACT tail instead of a long
                        # RoPE DVE tail.
                        if tt < 3:
                            emit_v()
                        for w_t, dstT in ((wq_t, qT_t), (wk_t, kT_t)):
                            for i, j, fo in ((0, 2, 0), (1, 3, 1)):
                                ps2 = []
                                for dc in (i, j):
                                    ps_t = pp.tile([128, 512], f32, tag="pp", name="pp")
                                    for e in range(8):
                                        nc.tensor.matmul(
                                            ps_t[:],
                                            w_t[:, e, dc * 128:(dc + 1) * 128],
                                            xt[:, e],
                                            start=(e == 0), stop=(e == 7))
                                    ps2.append(ps_t)
                                pi, pj = ps2
                                c_, s_ = cs[:, fo], sn[:, fo]
                                t0 = tp.tile([128, 512], f32, tag="rt", name="rt")
                                t1 = tp.tile([128, 512], f32, tag="rt", name="rt")
                                nc.vector.tensor_mul(t0[:], pi[:], c_)
                                nc.vector.tensor_mul(t1[:], pj[:], s_)
                                nc.vector.tensor_sub(
                                    dstT[i][:, s0:s0 + 512], t0[:], t1[:])
                                t2 = tp.tile([128, 512], f32, tag="rt", name="rt")
                                t3 = tp.tile([128, 512], f32, tag="rt", name="rt")
                                nc.vector.tensor_mul(t2[:], pi[:], s_)
                                nc.vector.tensor_mul(t3[:], pj[:], c_)
                                nc.vector.tensor_add(
                                    dstT[j][:, s0:s0 + 512], t2[:], t3[:])
                        if tt == 3:
                            emit_v()

                # ----- attention + o_proj phase (256-wide q tiles) -----
                with ExitStack() as actx:
                    ep = actx.enter_context(tc.tile_pool(name="ep", bufs=6))
                    atp = actx.enter_context(tc.tile_pool(name="atp", bufs=1))
                    ivp = actx.enter_context(tc.tile_pool(name="ivp", bufs=2))
                    obp = actx.enter_context(tc.tile_pool(name="obp", bufs=2))
                    ssp = actx.enter_context(tc.tile_pool(name="ssp", bufs=2))
                    # PSUM: matmul start=True zeroes the whole 2KB bank (the
                    # "zero region"), so every accumulator needs its own
                    # bank: 4 attn + 2 score + 2 shared o_proj/rowsum = 8.
                    scp = actx.enter_context(
                        tc.tile_pool(name="scp", bufs=2, space="PSUM"))
                    app = actx.enter_context(
                        tc.tile_pool(name="app", bufs=1, space="PSUM"))
                    opp = actx.enter_context(
                        tc.tile_pool(name="opp", bufs=2, space="PSUM"))

                    def emit_oproj(m):
                        # 1/rowsum is folded into the psum evacuation as a
                        # per-partition (per-token) scale. On the very last
                        # tile, evacuate+store in 256-wide pieces to shorten
                        # the serial tail after the final matmul.
                        q0 = m * QT
                        last = (b == 1 and m == NQT - 1)
                        for t4 in range(2):
                            ob = obp.tile([128, E], f32, tag="ob", name="ob")
                            r0 = tok0 + q0 + t4 * 128
                            for et in range(2):
                                op_ps = opp.tile([128, 512], f32, tag="op", name="op")
                                for dc in range(4):
                                    nc.tensor.matmul(
                                        op_ps[:],
                                        at_sb[m % 2][dc][:, t4 * 128:(t4 + 1) * 128],
                                        wo[dc][:, et * 512:(et + 1) * 512],
                                        start=(dc == 0), stop=(dc == 3))
                                nparts = 2 if (last and t4 == 1) else 1
                                w = 512 // nparts
                                for h in range(nparts):
                                    sl = slice(et * 512 + h * w, et * 512 + (h + 1) * w)
                                    if last and h == 1:
                                        nc.scalar.activation(
                                            ob[:, sl], op_ps[:, h * w:(h + 1) * w],
                                            AF.Copy,
                                            scale=inv_sb[m % 2][:, t4:t4 + 1])
                                    else:
                                        nc.vector.tensor_scalar_mul(
                                            ob[:, sl], op_ps[:, h * w:(h + 1) * w],
                                            inv_sb[m % 2][:, t4:t4 + 1])
                                    nc.sync.dma_start(out_d[r0:r0 + 128, sl],
                                                      ob[:, sl])

                    at_sb = {0: None, 1: None}
                    inv_sb = {0: None, 1: None}
                    for m in range(NQT):
                        q0 = m * QT
                        # off-diagonal 256-wide k-chunk ops; on the diagonal,
                        # chunk 2m runs full-width with an additive mask and
                        # chunk 2m+1 runs only its live 128-q half (its other
                        # half is fully above the diagonal).
                        ops = [(c, 0, QT, None) for c in range(2 * m)]
                        ops += [(2 * m, 0, QT, mk256), (2 * m + 1, 128, 128, mk128)]
                        nops = len(ops)
                        attn_ps = [app.tile([128, QT], f32, tag=f"attn{d}",
                                            name=f"attn{d}") for d in range(4)]
                        S = ssp.tile([128, QT], bf16, tag="S", name="S")

                        def emit_pv(exs, kc, qlo, qw, oi, nops=nops,
                                    attn_ps=attn_ps):
                            for dc in range(4):
                                nc.tensor.matmul(
                                    attn_ps[dc][:, qlo:qlo + qw],
                                    vv_chunks[kc][:, dc * 128:(dc + 1) * 128],
                                    exs,
                                    start=(oi == 0), stop=(oi == nops - 1))

                        pending = []
                        for oi, (kc, qlo, qw, mask) in enumerate(ops):
                            sc_t = scp.tile([128, QT], f32, tag="sc", name="sc")
                            sc_ps = sc_t[:, :qw]
                            for dc in range(4):
                                nc.tensor.matmul(
                                    sc_ps,
                                    kT_t[dc][:, kc * 128:(kc + 1) * 128],
                                    qT_t[dc][:, q0 + qlo:q0 + qlo + qw],
                                    start=(dc == 0), stop=(dc == 3))
                            if mask is not None:
                                nc.vector.tensor_add(sc_ps, sc_ps, mask)
                            ex = ep.tile([128, QT], bf16, tag="ex", name="ex")
                            exs = ex[:, :qw]
                            nc.scalar.activation(exs, sc_ps, AF.Exp, scale=SCALE)
                            if oi == 0:
                                nc.vector.tensor_copy(S[:], exs)
                            else:
                                nc.vector.tensor_add(S[:, qlo:qlo + qw],
                                                     S[:, qlo:qlo + qw], exs)
                            pending.append((exs, kc, qlo, qw, oi))
                            if len(pending) > 3:
                                emit_pv(*pending.pop(0))
                        for args in pending:
                            emit_pv(*args)
                        # transposed rowsum: rsT[q_local, t4] = sum_k S[k, q]
                        # via two 1-column matmuls (S halves as stationary),
                        # sharing one opp-pool bank (2nd accumulates into the
                        # bank zeroed by the 1st's start).
                        rs_full = opp.tile([128, 512], f32, tag="op", name="op")
                        nc.tensor.matmul(rs_full[:, 0:1], S[:, 0:128],
                                         onescol[:], start=True, stop=False)
                        nc.tensor.matmul(rs_full[:, 1:2], S[:, 128:256],
                                         onescol[:], start=False, stop=True)
                        inv2 = ivp.tile([128, 2], f32, tag="inv", name="inv")
                        nc.vector.reciprocal(inv2[:], rs_full[:, 0:2])
                        inv_sb[m % 2] = inv2
                        # psum evacuations stay on DVE: ACT work queued here
                        # delays the next tile's exps (FIFO), which stalls PV.
                        # Only the final tile (no exps after it) splits across
                        # ACT+DVE to pace the immediately-following o_proj.
                        at_sb[m % 2] = [
                            atp.tile([128, QT], bf16, tag=f"at{m % 2}_{dc}",
                                     name=f"at{m % 2}_{dc}")
                            for dc in range(4)]
                        for dc in range(4):
                            if b == 1 and m == NQT - 1 and dc % 2 == 1:
                                nc.scalar.copy(
                                    at_sb[m % 2][dc][:], attn_ps[dc][:])
                            else:
                                nc.vector.tensor_copy(
                                    at_sb[m % 2][dc][:], attn_ps[dc][:])
                        if m > 0:
                            emit_oproj(m - 1)
                        if b == 0:
                            # prefetch batch-1 inputs while the PE is busy:
                            # HAM drops to 4/8 if it ever idles at the
                            # batch transition.
                            if 2 <= m <= 5:
                                issue_x_dma(1, m - 2)
                            if m == 6:
                                issue_cs_dma(1, 0)
                            if m == 7:
                                issue_cs_dma(1, 1)
                    emit_oproj(NQT - 1)
    nc.compile()
    return nc


def _host_tables():
    inv_freq = 1.0 / (ROPE_BASE ** (np.arange(0, D, 2, dtype=np.float64) / D))
    ang = np.arange(T, dtype=np.float64)[:, None] * inv_freq[None, :]  # [T, D/2]
    cosdt = np.ascontiguousarray(np.cos(ang).T.astype(np.float32))     # [D/2, T]
    sindt = np.ascontiguousarray(np.sin(ang).T.astype(np.float32))
    kk = np.arange(128)[:, None]
    masks = np.empty((128, 384), dtype=np.float32)
    masks[:, 0:QT] = np.where(kk <= np.arange(QT)[None, :], 0.0, NEG)
    masks[:, QT:384] = np.where(kk <= np.arange(128)[None, :], 0.0, NEG)
    return cosdt, sindt, masks


def kernel(x, Wq, Wk, Wv, Wo):
    global LAST_RESULTS
    import ml_dtypes
    from concourse import bass_utils

    if "nc" not in _CACHE:
        _CACHE["nc"] = _build()
    nc = _CACHE["nc"]

    bf16 = ml_dtypes.bfloat16
    x = np.asarray(x, dtype=np.float32)
    Wq = np.asarray(Wq, dtype=np.float32)
    Wk = np.asarray(Wk, dtype=np.float32)
    Wv = np.asarray(Wv, dtype=np.float32)
    Wo = np.asarray(Wo, dtype=np.float32)

    xT = np.ascontiguousarray(x.reshape(NTOK, E).T).astype(bf16)  # [E, NTOK]
    cosdt, sindt, masks = _host_tables()

    in_maps = []
    for h in range(H):
        in_maps.append({
            "xT": xT,
            "wqT": np.ascontiguousarray(Wq[h * D:(h + 1) * D, :].T).astype(bf16),
            "wkT": np.ascontiguousarray(Wk[h * D:(h + 1) * D, :].T).astype(bf16),
            "wvT": np.ascontiguousarray(Wv[h * D:(h + 1) * D, :].T).astype(bf16),
            "woT": np.ascontiguousarray(Wo[:, h * D:(h + 1) * D].T).astype(bf16),
            "cosdt": cosdt,
            "sindt": sindt,
            "masks": masks,
        })

    kwargs = {}
    if PROFILE:
        import sys
        import types
        import trn_agent_boot.trn_boot as _tb
        hook = _tb._ntff_profile_via_ctypes("/opt/axon/libaxon_pjrt.so")
        mod = types.ModuleType("antenv.axon_hooks")
        mod.get_axon_ntff_profile_hook = lambda: hook
        mod.set_axon_ntff_profile_hook = lambda h_: None
        sys.modules["antenv.axon_hooks"] = mod
        bass_utils.upload_artifacts = lambda tmpdir: tmpdir
        kwargs = dict(trace=True, trace_cores=[0])

    res = bass_utils.run_bass_kernel_spmd(
        nc, in_maps, core_ids=list(range(H)), **kwargs)
    LAST_RESULTS = res

    out = res.results[0]["out"].astype(np.float32).copy()
    for h in range(1, H):
        out += res.results[h]["out"]
    return out.reshape(B, T, E)
